# revision 8
# baseline (speedup 1.0000x reference)
# Multi-head attention (B=2, S=2048, E=1024, H=16, D=64) on 8 NeuronCores.
#
# Sharding: core c -> (batch b = c//4, head-group g = c%4 of 4 heads).
#   - qkv_proj column-parallel per head group, out_proj row-parallel.
#   - Each core computes a partial [S, E] output (its heads' contribution);
#     host sums the 4 partials per batch and adds b_out (the unshard).
#
# Per-core kernel (all matmul inputs bf16, fp32 PSUM accumulation):
#   qT/kT   [d, s] via w^T x^T matmuls; qk-bias added on DVE during the
#           PSUM->SBUF copy; v-bias folded into b_out on the host
#   scoresT [j, i] per head = kT(lhsT) @ qT(rhs), K=64 -- the two heads run
#           CONCURRENTLY on the PE via row-tiling (partition bases 0/64;
#           measured: 2nd matmul of each pair ~3ns), so scores cost only
#           ~220ns/step, not 2x216
#   exp on ScalarE with fused 1/sqrt(d) scale into a 16-step bf16 SBUF ring
#           (no max subtraction: scores are small, exp cannot overflow)
#   PV: v augmented with a ones column -> pv[65, 512] per head; row 64 =
#           softmax denominator; PV runs DEF=12 steps behind the exp stream
#   normalize: DVE approx reciprocal + Pool partition_broadcast + DVE mul
#   out_proj: head-pairs packed -> k=128 matmuls, partial out in PSUM
#   Flat software-pipelined stream paced against the dual drumbeat:
#   ACT exp ~1060ns/step vs PE ~1080ns/step (sc 220 + pv 432 + injected
#   projection/out-proj filler ~430). fp8 DoubleRow PV was tried and
#   REVERTED: on this silicon DR matmuls stream at ~320ns per 512 cols
#   (dual-plane ldweights don't overlap), costlier than bf16 PV.

import numpy as np

import concourse.bacc as bacc
import concourse.bass as bass
import concourse.mybir as mybir
import concourse.tile as tile
from concourse.bass_utils import run_bass_kernel_spmd

B, S, E = 2, 2048, 1024
H_TOT, D = 16, 64
HG = 4                  # heads per core
GD = HG * D             # 256 group dim
N_CORES = 8
P = 128
EO = E // P             # 8 contraction tiles
NB_QK = 2 * GD // P     # 4 n-blocks for [q, k]
SB = S // P             # 16 s/j blocks
FP32 = mybir.dt.float32
BF16 = mybir.dt.bfloat16
SCALE = float(D) ** -0.5
RING = 16               # pexp ring depth in steps

_NC_CACHE = None


def _build_program() -> bass.Bass:
    nc = bacc.Bacc(trn_type="TRN2")
    xT = nc.dram_tensor("xT", [4, P, EO, 512], BF16, kind="ExternalInput")
    w_qk = nc.dram_tensor("w_qk", [P, EO, 2 * GD], BF16, kind="ExternalInput")
    w_v = nc.dram_tensor("w_v", [P, EO, GD], BF16, kind="ExternalInput")
    b_qk = nc.dram_tensor("b_qk", [P, NB_QK], FP32, kind="ExternalInput")
    w_o = nc.dram_tensor("w_o", [P, 2, E], BF16, kind="ExternalInput")
    out = nc.dram_tensor("out", [S, E], FP32, kind="ExternalOutput")

    with tile.TileContext(nc) as tc:
        _emit(tc, xT, w_qk, w_v, b_qk, w_o, out)
    nc.finalize()
    return nc


def _emit(tc, xT, w_qk, w_v, b_qk, w_o, out):
    nc = tc.nc
    Exp = mybir.ActivationFunctionType.Exp
    Add = mybir.AluOpType.add

    with (
        tc.tile_pool(name="persist", bufs=1) as persist,
        tc.tile_pool(name="stage", bufs=2) as stage,
        tc.tile_pool(name="out_pool", bufs=3) as out_pool,
        tc.tile_pool(name="ps_mm", bufs=2, space="PSUM") as ps_mm,
        tc.tile_pool(name="ps_sc", bufs=2, space="PSUM") as ps_sc,
        tc.tile_pool(name="ps_pv", bufs=2, space="PSUM") as ps_pv,
    ):
        # ---------------- load inputs (host pre-cast to bf16) ----------------
        # Order = first-use order; one big DMA per logical tensor chunk (each
        # trigger costs ~600ns of Sync time); wqk/xT-ic0 split in eo-halves so
        # the first qkT matmuls chase the half-landings.
        wqk_sb = persist.tile([P, EO, 2 * GD], BF16)
        xT_sb = persist.tile([P, 4, EO, 512], BF16)
        for h in range(2):
            nc.sync.dma_start(
                wqk_sb[:, 4 * h:4 * h + 4, :], w_qk[:, 4 * h:4 * h + 4, :]
            )
            nc.sync.dma_start(
                xT_sb[:, 0, 4 * h:4 * h + 4, :], xT[0, :, 4 * h:4 * h + 4, :]
            )
        bqk_sb = persist.tile([P, NB_QK], FP32)
        nc.sync.dma_start(bqk_sb, b_qk[:, :])
        wv_sb = persist.tile([P, EO, GD], BF16)
        nc.sync.dma_start(wv_sb[:, :, :], w_v[:, :, :])
        for ic in range(1, 4):
            nc.sync.dma_start(xT_sb[:, ic, :, :], xT[ic, :, :, :])
        wo_sb = persist.tile([P, 2, E], BF16)
        nc.sync.dma_start(wo_sb[:, :, :], w_o[:, :, :])

        # Warm the ACT exp table before the attention phase needs it.
        act_warm = persist.tile([1, D], FP32)
        nc.vector.memset(act_warm, 1.0)
        act_dummy = persist.tile([1, D], FP32)
        nc.scalar.activation(act_dummy, act_warm, Exp)

        # ---------------- persistent activations ----------------
        # qkT layout: n-blocks [q01, q23, k01, k23]; rows 0-63 even head, 64-127 odd
        qkT_sb = persist.tile([P, NB_QK, S], BF16)
        # pexp ring: [j, step-slot, head-in-pair, i]; exp(k) -> slot k%RING
        ring_sb = persist.tile([P, RING, 2, 512], BF16)
        # v augmented with ones column: [j, s-block, head, D+1]
        vaug_sb = persist.tile([P, SB, HG, D + 1], BF16)
        nc.vector.memset(vaug_sb[:, :, :, D], 1.0)
        attnT_sb = persist.tile([P, 2, S], BF16)

        qkT_ps = {}

        def emit_qkT_half(nb, ic, half):
            # qkT[n-block nb, s-chunk ic] = w_qk_nb^T x^T; bias added on DVE
            # during the PSUM->SBUF copy. Split in two ~0.85us halves so one
            # injected burst never outlasts the sc/exp lookahead buffer.
            if half == 0:
                qkT_ps[(nb, ic)] = ps_mm.tile([P, 512], FP32, tag="ps", name="ps_qk")
            ps = qkT_ps[(nb, ic)]
            for eo in range(4 * half, 4 * half + 4):
                nc.tensor.matmul(
                    ps,
                    lhsT=wqk_sb[:, eo, nb * P:(nb + 1) * P],
                    rhs=xT_sb[:, ic, eo, :],
                    start=(eo == 0), stop=(eo == EO - 1),
                )
            if half == 1:
                del qkT_ps[(nb, ic)]
                nc.vector.tensor_scalar(
                    qkT_sb[:, nb, ic * 512:(ic + 1) * 512],
                    ps,
                    bqk_sb[:, nb:nb + 1],
                    None,
                    Add,
                )

        def emit_v(sb):
            # v[s-block sb, :] for all heads; v-bias folded into b_out on the
            # host (commutes through softmax).
            psf = ps_mm.tile([P, 512], FP32, tag="ps", name="ps_v")
            psv = psf[:, :GD]
            for eo in range(EO):
                nc.tensor.matmul(
                    psv,
                    lhsT=xT_sb[:, sb // 4, eo, (sb % 4) * P:(sb % 4 + 1) * P],
                    rhs=wv_sb[:, eo, :],
                    start=(eo == 0), stop=(eo == EO - 1),
                )
            nc.vector.tensor_copy(
                vaug_sb[:, sb, :, 0:D], psv.rearrange("p (h d) -> p h d", d=D)
            )

        def emit_outproj_piece(icq, piece, pool=None, tag="ps"):
            # one [128 s, 512 e] block of the partial out rows for i-chunk icq
            sb2, nck = piece // 2, piece % 2
            s0 = icq * 512 + sb2 * P
            po = (pool or ps_mm).tile([P, 512], FP32, tag=tag, name="ps_o")
            for pair in range(2):
                nc.tensor.matmul(
                    po,
                    lhsT=attnT_sb[:, pair, s0:s0 + P],
                    rhs=wo_sb[:, pair, nck * 512:(nck + 1) * 512],
                    start=(pair == 0), stop=(pair == 1),
                )
            ot = out_pool.tile([P, 512], FP32, tag="ot")
            nc.vector.tensor_copy(ot, po)
            nc.sync.dma_start(out[s0:s0 + P, nck * 512:(nck + 1) * 512], ot)

        # ---------------- attention stream primitives ----------------
        def emit_sc_exp(k):
            ci, jb = divmod(k, SB)
            icq, pr = ci // 2, ci % 2
            i0 = icq * 512
            sc = ps_sc.tile([P, 1024], FP32, tag="sc")
            nc.tensor.matmul(
                sc[:, 0:512],
                lhsT=qkT_sb[0:D, 2 + pr, jb * P:(jb + 1) * P],
                rhs=qkT_sb[0:D, pr, i0:i0 + 512],
                start=True, stop=True,
            )
            nc.tensor.matmul(
                sc[:, 512:1024],
                lhsT=qkT_sb[D:2 * D, 2 + pr, jb * P:(jb + 1) * P],
                rhs=qkT_sb[D:2 * D, pr, i0:i0 + 512],
                start=True, stop=True,
            )
            nc.scalar.activation(
                ring_sb[:, k % RING, :, :], sc, Exp, scale=SCALE
            )

        pv_ps = {}
        pv_sbs = {}

        def emit_pv(k):
            ci, jb = divmod(k, SB)
            slot = k % RING
            if jb == 0:
                pv_ps[(ci, 0)] = ps_pv.tile([D + 1, 512], FP32, tag="pv", name="pvA")
                pv_ps[(ci, 1)] = ps_pv.tile([D + 1, 512], FP32, tag="pv", name="pvB")
            for hl in range(2):
                nc.tensor.matmul(
                    pv_ps[(ci, hl)],
                    lhsT=vaug_sb[:, jb, 2 * (ci % 2) + hl, :],
                    rhs=ring_sb[:, slot, hl, :],
                    start=(jb == 0), stop=(jb == SB - 1),
                )
            if jb == SB - 1:
                # drain immediately behind the stopping matmuls so the PSUM
                # banks free before the next chunk's chains allocate them
                for hl in range(2):
                    pv = pv_ps.pop((ci, hl))
                    vsb = stage.tile([D + 1, 512], FP32, tag="pvsb", bufs=4)
                    nc.vector.tensor_copy(vsb, pv)
                    pv_sbs[(ci, hl)] = vsb

        def finish_head(ci, hl):
            # normalize: rebase denom row to partition 0 (approx recip can't
            # cross partition bases), recip on DVE, broadcast on Pool, mul
            icq, pr = ci // 2, ci % 2
            i0 = icq * 512
            vsb = pv_sbs.pop((ci, hl))
            dcp = stage.tile([1, 512], FP32, tag="denom", bufs=4)
            nc.vector.tensor_copy(dcp, vsb[D:D + 1, :])
            recip = stage.tile([1, 512], FP32, tag="recip", bufs=4)
            nc.vector.reciprocal_approx_fast(recip, dcp)
            bc = stage.tile([D, 512], FP32, tag="bcsb", bufs=2)
            nc.gpsimd.partition_broadcast(bc, recip)
            nc.vector.tensor_mul(
                attnT_sb[hl * D:(hl + 1) * D, pr, i0:i0 + 512], vsb[0:D, :], bc
            )
            tt_done[icq] = tt_done.get(icq, 0) + 1

        # ---------------- prologue: only what score-block 0 needs ----------
        emit_qkT_half(2, 0, 0); emit_qkT_half(0, 0, 0)
        emit_qkT_half(2, 0, 1); emit_qkT_half(0, 0, 1)

        # Filler injection: v-projections and remaining qkT chains ride the
        # attention steps, paced against the ACT exp drumbeat.
        inject = {}

        def add_inject(s, fn):
            inject.setdefault(s, []).append(fn)

        for sb in range(SB):
            add_inject(sb + 8, (lambda b: lambda: emit_v(b))(sb))

        def add_qkT(s, nb, ic):
            add_inject(s, lambda: emit_qkT_half(nb, ic, 0))
            add_inject(s + 1, lambda: emit_qkT_half(nb, ic, 1))

        add_qkT(0, 2, 1)     # k01-ic1: sc(4) emitted at step 2
        add_qkT(3, 2, 2)     # k01-ic2: sc(8) at step 6
        add_qkT(6, 2, 3)     # k01-ic3: sc(12) at step 10
        add_qkT(8, 3, 0)     # k23-ic0: sc(16) at step 14
        add_qkT(10, 1, 0)    # q23-ic0: sc(16) at step 14
        add_qkT(14, 3, 1)    # k23-ic1: sc(20) at step 18
        add_qkT(16, 3, 2)    # k23-ic2: sc(24) at step 22
        add_qkT(18, 3, 3)    # k23-ic3: sc(28) at step 26
        add_qkT(22, 0, 1)    # q01-ic1: sc(32) at step 30
        add_qkT(40, 1, 1)    # q23-ic1: sc(48) at step 46
        add_qkT(44, 0, 2)    # q01-ic2: sc(64) at step 62
        add_qkT(53, 1, 2)    # q23-ic2: sc(80) at step 78
        add_qkT(69, 0, 3)    # q01-ic3: sc(96) at step 94
        add_qkT(85, 1, 3)    # q23-ic3: sc(112) at step 110

        # ---------------- flat software-pipelined stream ----------------
        LOOKAHEAD = 2
        DEF = 12             # PV stream offset behind the sc/exp stream
        NSTEP = 8 * SB
        TAIL = 18
        pending_finish = []
        pending_outproj = []
        tt_done = {}
        # out-proj piece slots: icq pieces unlocked by the 4 normalize muls
        oslot_steps = set()
        for icq in range(3):
            for idx in range(8):
                oslot_steps.add(32 * icq + 46 + 2 * idx)

        for k in range(LOOKAHEAD):
            emit_sc_exp(k)
        for s in range(NSTEP + TAIL):
            if s + LOOKAHEAD < NSTEP:
                emit_sc_exp(s + LOOKAHEAD)
            while pending_finish:
                finish_head(*pending_finish.pop(0))
            t = s - DEF
            if 0 <= t < NSTEP:
                emit_pv(t)
                if t % 16 == 15:
                    pending_finish.append((t // 16, 0))
                    pending_finish.append((t // 16, 1))
            for fn in inject.get(s, ()):
                fn()
            if s in oslot_steps and pending_outproj:
                icq_o, piece = pending_outproj[0]
                if tt_done.get(icq_o, 0) == 4:
                    pending_outproj.pop(0)
                    emit_outproj_piece(icq_o, piece)
            if s % 32 == 13 and (s - 13) // 32 < 3:
                pending_outproj.extend(
                    ((s - 13) // 32, piece) for piece in range(8)
                )

        # ---------------- epilogue: i-chunk 3 out-proj ----------------
        # pair-0 halves first (only need ci6 heads, done before ci7's PV
        # lands), rotated across all three PSUM pools; pair-1 + copy + DMA
        # follow once the last normalize muls land.
        pools3 = ((ps_mm, "ps"), (ps_sc, "sc"), (ps_pv, "pv"))
        epi = []
        for idx, piece in enumerate(range(8)):
            sb2, nck = piece // 2, piece % 2
            s0 = 3 * 512 + sb2 * P
            po = None
            if idx < 6:
                pool, tag = pools3[idx % 3]
                po = pool.tile([P, 512], FP32, tag=tag, name="ps_o")
                nc.tensor.matmul(
                    po,
                    lhsT=attnT_sb[:, 0, s0:s0 + P],
                    rhs=wo_sb[:, 0, nck * 512:(nck + 1) * 512],
                    start=True, stop=False,
                )
            epi.append((piece, po))
        for idx, (piece, po) in enumerate(epi):
            sb2, nck = piece // 2, piece % 2
            s0 = 3 * 512 + sb2 * P
            if po is None:
                pool, tag = pools3[idx % 3]
                po = pool.tile([P, 512], FP32, tag=tag, name="ps_o")
                nc.tensor.matmul(
                    po,
                    lhsT=attnT_sb[:, 0, s0:s0 + P],
                    rhs=wo_sb[:, 0, nck * 512:(nck + 1) * 512],
                    start=True, stop=False,
                )
            nc.tensor.matmul(
                po,
                lhsT=attnT_sb[:, 1, s0:s0 + P],
                rhs=wo_sb[:, 1, nck * 512:(nck + 1) * 512],
                start=False, stop=True,
            )
            ot = out_pool.tile([P, 512], FP32, tag="ot")
            nc.vector.tensor_copy(ot, po)
            nc.sync.dma_start(out[s0:s0 + P, nck * 512:(nck + 1) * 512], ot)


def _get_nc() -> bass.Bass:
    global _NC_CACHE
    if _NC_CACHE is None:
        _NC_CACHE = _build_program()
    return _NC_CACHE


def make_in_maps(x, w_qkv, b_qkv, w_out):
    import ml_dtypes

    bf16 = ml_dtypes.bfloat16
    x = np.asarray(x, dtype=np.float32)
    w_qkv = np.asarray(w_qkv, dtype=np.float32)
    b_qkv = np.asarray(b_qkv, dtype=np.float32)
    w_out = np.asarray(w_out, dtype=np.float32)

    in_maps = []
    for c in range(N_CORES):
        b, g = c // 4, c % 4
        q0 = g * GD
        # [4 ic, 128 p, 8 eo, 512] so every DMA chunk is contiguous
        xT_b = np.ascontiguousarray(
            x[b].T.astype(bf16).reshape(EO, P, 4, 512).transpose(2, 1, 0, 3)
        )
        w_qk_c = np.ascontiguousarray(
            np.concatenate(
                [w_qkv[:, q0:q0 + GD], w_qkv[:, E + q0:E + q0 + GD]], axis=1
            ).astype(bf16).reshape(EO, P, 2 * GD).transpose(1, 0, 2)
        )                                                          # [P, EO, 2*GD]
        w_v_c = np.ascontiguousarray(
            w_qkv[:, 2 * E + q0:2 * E + q0 + GD].astype(bf16)
            .reshape(EO, P, GD).transpose(1, 0, 2)
        )
        # b_qk as [128, NB_QK]: partition p of n-block nb holds bias for
        # qk-dim nb*128+p (per-partition scalar add on the qkT copy).
        b_qk_c = np.ascontiguousarray(
            np.concatenate([b_qkv[q0:q0 + GD], b_qkv[E + q0:E + q0 + GD]])
            .astype(np.float32).reshape(NB_QK, P).T
        )
        w_o_c = np.ascontiguousarray(
            w_out[q0:q0 + GD, :].astype(bf16).reshape(2, P, E).transpose(1, 0, 2)
        )                                                          # [P, 2, E]
        in_maps.append(
            {
                "xT": xT_b,
                "w_qk": w_qk_c,
                "w_v": w_v_c,
                "b_qk": b_qk_c,
                "w_o": w_o_c,
            }
        )
    return in_maps


def unshard(results, b_qkv, w_out, b_out):
    # v-bias commutes through softmax-weighted averaging (weights sum to 1),
    # so its contribution is the constant row b_v @ w_out, folded in here.
    b_out = np.asarray(b_out, dtype=np.float32)
    b_v = np.asarray(b_qkv, dtype=np.float32)[2 * E:]
    b_eff = b_out + b_v @ np.asarray(w_out, dtype=np.float32)
    out = np.empty((B, S, E), dtype=np.float32)
    for b in range(B):
        acc = results[4 * b]["out"].astype(np.float32, copy=True)
        for g in range(1, 4):
            acc += results[4 * b + g]["out"]
        out[b] = acc + b_eff
    return out


def kernel(x, w_qkv, b_qkv, w_out, b_out):
    in_maps = make_in_maps(x, w_qkv, b_qkv, w_out)
    res = run_bass_kernel_spmd(_get_nc(), in_maps, core_ids=list(range(N_CORES)))
    return unshard(res.results, b_qkv, w_out, b_out)
